# revision 5
# baseline (speedup 1.0000x reference)
"""BoxFilter kernel for Trainium2 (8 NeuronCores) — bf16 I/O, v5.

out[b,0,i,j] = sum_c sum_{|di|<=15,|dj|<=15} x[b,c,i+di,j+dj] (edge-clamped),
matching the reference cumsum+shifted-diff formulation (separable box sums).

Sharding: data-parallel over (batch, H-half) -> 8 shards, 1024 output rows
per core.

v5 layout: the host builds a per-core slab with (h, c)-interleaved rows
  slab[3*j + c, :] = x[b, c, h0 + j - 16, :]   (zeros outside the image)
so the vertical 31-tap box *and* the channel sum are ONE set of band
matmuls: output tile t (128 rows) contracts slab tiles 3t..3t+3 with four
constant 0/1 band matrices (93 taps per output row). Every DMA is a single
contiguous 512KB [128, 2048] transfer.

Per-core pipeline per 128-row output tile (HW-measured rates):
  - 25 input-tile loads issued upfront: sync queue (13) + gpsimd SWDGE (12);
    all tiles stay resident in SBUF (no reuse waits)
  - vertical box + channel sum: 16 accumulating bf16 matmuls (4 bands x 4
    512-col PSUM banks), weight-grouped; 8 PSUM banks double-buffer tiles
  - ACT copies PSUM (f32) into the zero-padded xp tile (pads zeroed once)
  - horizontal 31-tap box: one DVE tensor_tensor_scan per tile (fp32 state,
    bf16 out). DVE scan rate ~2.15ns/elem is the kernel's ~35.5us floor;
    DVE does nothing else.
  - stores on the SWDGE queue; first/last tiles use chained half-scans so
    the first store departs early / pipeline drains early
  - PE pre-warmed with dummy matmuls during the DMA fill (p-state ramp)
"""

import numpy as np
import ml_dtypes

BF = ml_dtypes.bfloat16

R = 15
TAP = 2 * R + 1          # 31
B, C, H, W = 4, 3, 2048, 2048
HALF = H // 2            # 1024 output rows per core
S_ROWS = HALF + 32       # 1056 h-rows per core (16-row halo each side)
SLAB_ROWS = 3 * S_ROWS   # 3168 interleaved (h, c) rows
SLAB_PAD = 3200          # padded to 25 full 128-row tiles (pad rows zero)
N_CORES = 8
P = 128
N_OUT_TILES = HALF // P  # 8
N_IN_TILES = SLAB_PAD // P  # 25
MM_N = 512               # one PSUM bank
PAD_L = TAP              # left zero pad for the scan (31)
XP_W = PAD_L + W + R     # 2094
SCAN_N = W + R           # 2063 scan steps; out col j = scan[j + R]
N_BANKS = W // MM_N      # 4

_CACHE = {}


def _band_matrices():
    # out row i of tile t needs slab rows 3i+3 .. 3i+95 (window-relative;
    # window = slab rows [384t, 384t+512) = slab tiles 3t..3t+3).
    # band_k[r, i] = 1 iff (128k + r) // 3 in [i+1, i+31].  0/1 exact in bf16.
    r = np.arange(P)[:, None]
    i = np.arange(P)[None, :]
    bands = []
    for k in range(4):
        j = (128 * k + r) // 3
        bands.append(((j >= i + 1) & (j <= i + TAP)).astype(BF))
    return np.concatenate(bands, axis=1)  # [P, 4P], band k at cols 128k:


def _build_kernel(tc, nc, out, xs, bands_d, mybir, bass):
    from contextlib import ExitStack

    f32 = mybir.dt.float32
    bf16 = mybir.dt.bfloat16
    add = mybir.AluOpType.add
    sub = mybir.AluOpType.subtract

    with ExitStack() as ctx:
        const_pool = ctx.enter_context(tc.tile_pool(name="const", bufs=1))
        xc_pool = ctx.enter_context(tc.tile_pool(name="xc", bufs=1))
        xp_pool = ctx.enter_context(tc.tile_pool(name="xp", bufs=1))
        box_pool = ctx.enter_context(tc.tile_pool(name="box", bufs=3))
        psum_pool = ctx.enter_context(
            tc.tile_pool(name="psum", bufs=2, space=bass.MemorySpace.PSUM)
        )

        bands = const_pool.tile([P, 4 * P], bf16)
        nc.sync.dma_start(bands[:], bands_d)

        # keep the PE p-state clock ramping while the first DMAs land
        wps = psum_pool.tile([P, MM_N], f32, name="ps0")
        for _ in range(24):
            nc.tensor.matmul(wps[:, 0:P], bands[:, 0:P], bands[:, 0:P],
                             start=True, stop=True, skip_group_check=True)

        # persistent xp buffers: zero pads once, rotate manually
        N_XP = 4
        xps = [xp_pool.tile([P, XP_W], f32, name=f"xp{i}") for i in range(N_XP)]
        for x_ in xps:
            nc.gpsimd.memset(x_[:, 0:PAD_L], 0.0)
            nc.gpsimd.memset(x_[:, PAD_L + W : XP_W], 0.0)

        # all 25 input-tile loads upfront, alternating sync / gpsimd queues;
        # neither engine has dependent work queued ahead, so issues drain
        # back-to-back and the queues stay ~3 tiles ahead of the PE.
        xcs = []
        for u in range(N_IN_TILES):
            xc = xc_pool.tile([P, W], bf16, name=f"xc{u}")
            eng = nc.sync if u % 2 == 0 else nc.gpsimd
            eng.dma_start(xc[:], xs[P * u : P * (u + 1), :])
            xcs.append(xc)

        for t in range(N_OUT_TILES):
            xp = xps[t % N_XP]

            # vertical box + channel sum: accumulate 4 band matmuls per
            # 512-col PSUM bank, grouped by stationary weight (band k reads
            # only slab tile 3t+k, so the fill loads unblock bands in order)
            psums = [psum_pool.tile([P, MM_N], f32, name=f"ps{nb}")
                     for nb in range(N_BANKS)]
            for k in range(4):
                band = bands[:, P * k : P * (k + 1)]
                src = xcs[3 * t + k]
                for nb in range(N_BANKS):
                    cs = slice(MM_N * nb, MM_N * (nb + 1))
                    nc.tensor.matmul(
                        psums[nb][:], band, src[:, cs],
                        start=(k == 0), stop=(k == 3),
                    )
            for nb in range(N_BANKS):
                nc.scalar.copy(
                    xp[:, PAD_L + MM_N * nb : PAD_L + MM_N * (nb + 1)],
                    psums[nb][:],
                )

            box = box_pool.tile([P, SCAN_N], bf16)
            if t == 0:
                # ramp tile: chained half-scans split at col 1023 so the
                # first half depends only on ACT banks 0-1
                H0 = 1023
                with tc.high_priority():
                    nc.vector.tensor_tensor_scan(
                        box[:, 0:H0],
                        xp[:, PAD_L : PAD_L + H0],
                        xp[:, 0:H0],
                        0.0,
                        add,
                        sub,
                    )
                nc.gpsimd.dma_start(
                    out[P * t : P * (t + 1), 0 : H0 - R], box[:, R:H0])
                with tc.high_priority():
                    nc.vector.tensor_tensor_scan(
                        box[:, H0:SCAN_N],
                        xp[:, PAD_L + H0 : PAD_L + SCAN_N],
                        xp[:, H0:SCAN_N],
                        box[:, H0 - 1 : H0],
                        add,
                        sub,
                    )
                nc.gpsimd.dma_start(
                    out[P * t : P * (t + 1), H0 - R : W],
                    box[:, H0 : R + W])
            elif t < N_OUT_TILES - 1:
                with tc.high_priority():
                    nc.vector.tensor_tensor_scan(
                        box[:],
                        xp[:, PAD_L : PAD_L + SCAN_N],
                        xp[:, 0:SCAN_N],
                        0.0,
                        add,
                        sub,
                    )
                nc.gpsimd.dma_start(
                    out[P * t : P * (t + 1), :], box[:, R : R + W])
            else:
                # last tile: chained half-scans so the first half-store
                # departs ~2us before the second half finishes
                HN = SCAN_N // 2 + 8
                with tc.high_priority():
                    nc.vector.tensor_tensor_scan(
                        box[:, 0:HN],
                        xp[:, PAD_L : PAD_L + HN],
                        xp[:, 0:HN],
                        0.0,
                        add,
                        sub,
                    )
                nc.gpsimd.dma_start(
                    out[P * t : P * (t + 1), 0 : HN - R], box[:, R:HN])
                with tc.high_priority():
                    nc.vector.tensor_tensor_scan(
                        box[:, HN:SCAN_N],
                        xp[:, PAD_L + HN : PAD_L + SCAN_N],
                        xp[:, HN:SCAN_N],
                        box[:, HN - 1 : HN],
                        add,
                        sub,
                    )
                nc.sync.dma_start(
                    out[P * t : P * (t + 1), HN - R : W],
                    box[:, HN : R + W])


def _get_nc():
    if "nc" in _CACHE:
        return _CACHE["nc"]
    import concourse.bass as bass
    import concourse.tile as tile
    from concourse import bacc, mybir

    nc = bacc.Bacc(
        "TRN2", target_bir_lowering=False, debug=False, num_devices=N_CORES
    )
    bf16 = mybir.dt.bfloat16
    xs = nc.dram_tensor("xs", [SLAB_PAD, W], bf16, kind="ExternalInput")
    bd = nc.dram_tensor("bands", [P, 4 * P], bf16, kind="ExternalInput")
    out = nc.dram_tensor("out", [HALF, W], bf16, kind="ExternalOutput")

    with tile.TileContext(nc) as tc:
        _build_kernel(tc, nc, out.ap(), xs.ap(), bd.ap(), mybir, bass)
    nc.compile()
    _CACHE["nc"] = nc
    return nc


def _in_maps(x):
    bands = _band_matrices()
    xb = x.astype(BF)
    maps = []
    for k in range(N_CORES):
        b, half = divmod(k, 2)
        h0 = half * HALF
        lo = h0 - 16  # global image row of slab h-row 0
        g0, g1 = max(lo, 0), min(h0 + HALF + 16, H)
        xs = np.zeros((SLAB_PAD, W), BF)
        v = xb[b, :, g0:g1, :]                        # [C, n, W]
        v = np.ascontiguousarray(v.transpose(1, 0, 2)).reshape(-1, W)
        xs[3 * (g0 - lo) : 3 * (g0 - lo) + v.shape[0], :] = v
        maps.append({"xs": xs, "bands": bands})
    return maps


def _run(x, trace=False, tmpdir=None):
    from concourse.bass_utils import run_bass_kernel_spmd

    nc = _get_nc()
    res = run_bass_kernel_spmd(
        nc, _in_maps(x), list(range(N_CORES)), trace=trace, tmpdir=tmpdir
    )
    out = np.empty((B, 1, H, W), np.float32)
    for k in range(N_CORES):
        b, half = divmod(k, 2)
        out[b, 0, half * HALF : (half + 1) * HALF, :] = (
            res.results[k]["out"].astype(np.float32)
        )
    return out, res


def kernel(x: np.ndarray) -> np.ndarray:
    x = np.ascontiguousarray(x, dtype=np.float32)
    assert x.shape == (B, C, H, W)
    return _run(x)[0]
